# revision 8
# baseline (speedup 1.0000x reference)
"""Trainium2 Bass kernel for MultiHeadDifferentialAttention (v2).

Strategy: data-parallel over batch. B=8 batches map 1:1 onto the 8
NeuronCores; each core runs the full per-batch pipeline (QKV proj ->
differential attention -> LayerNorm -> output proj) with no collectives.

v2 restructure vs v1: the attention-value matmul keeps V as the
STATIONARY operand (e streams as the moving operand in 512-col runs),
so the AV phase runs at the N=512 streaming floor instead of paying a
weight-load per 129-col matmul, and the output lands directly in the
[channel, token] layout the final projection consumes (no transpose
phase). The softmax denominators and the LayerNorm-over-channels are
recovered via a colsum trick:

  - Wv is column-CENTERED per head on the host and the constant c=1/128
    is added on-device, so each key's value row sums to exactly 1.
    Then colsum_d(u~) == softmax denominator d[n], and the uniform
    shift c*d[n] the offset adds to u~ is annihilated by the LN's mean
    subtraction (exact, not approximate).
  - LN statistics over channels (= partitions here) come from
    ones-stationary matmuls (colsums), whose outputs are broadcast
    across all 128 partitions for free. LN is computed in scale-
    invariant form on z'' = 2^-s*(u1 - lam*(d1/d2)*u2) with the eps
    correction eps*(2^-s*d1)^2 folded into the variance, which makes
    the result EXACTLY equal to LN(a1@v - lam*a2@v).
  - rsqrt = exp(-0.5*ln(V)) on ScalarE; the activation-table patch pins
    exp and ln to one table set so no table reloads occur.

PSUM budget: scores 2x[128,2,512] (4 banks) + u~ 2x[128,512] (2) +
stats 2x[128,512] (2) = 8 banks exactly.
"""

import numpy as np

B, N, C, H = 8, 1024, 768, 12
D = C // H  # 64
TD = 2 * D  # 128
LAMBDA_INIT = 0.8 - 0.6 * np.exp(-0.3 * (1 - 1))  # 0.2
OUT_SCALE = 1.0 - LAMBDA_INIT  # 0.8
EPS = 1e-5
SCALE = float(D) ** -0.5  # 1/8
CONE = 1.0 / 128  # ones-offset folded into Wv rows
SSH = 3  # z'' = 2^-SSH * (u1 - rho*u2) keeps z''^2 in fp16 range
S2 = 2.0 ** -SSH

_BUILD_CACHE = {}
LAST_EXEC_NS = None


def _patch_act_tables(mybir, bacc):
    """Pin Exp and Ln to natural_log_exp_and_others so interleaving them
    never reloads the ScalarE spline tables."""
    from concourse import hw_specs

    orig = hw_specs.get_activation_tables
    if getattr(bacc.get_activation_tables, "_nlx_pinned", False):
        return

    def patched(arch):
        tables = orig(arch)
        exp = mybir.ActivationFunctionType.Exp
        ln = mybir.ActivationFunctionType.Ln
        for name, funcs in tables.items():
            if name != "natural_log_exp_and_others":
                funcs.discard(exp)
                funcs.discard(ln)
        return tables

    patched._nlx_pinned = True
    bacc.get_activation_tables = patched


def _build(lam: float, dbg: bool = False):
    import concourse.bass as bass  # noqa: F401
    import concourse.mybir as mybir
    import concourse.tile as tile
    from concourse import bacc

    _patch_act_tables(mybir, bacc)

    f32 = mybir.dt.float32
    f16 = mybir.dt.float16
    AF = mybir.ActivationFunctionType
    OP = mybir.AluOpType

    nc = bacc.Bacc(None, target_bir_lowering=False, debug=False)

    XT = nc.declare_dram_parameter("xT", [128, 6, 1024], f16, isOutput=False)
    WQR = nc.declare_dram_parameter("WqR", [12, 128, 6, 128], f16, isOutput=False)
    WKR = nc.declare_dram_parameter("WkR", [12, 128, 6, 128], f16, isOutput=False)
    WVR = nc.declare_dram_parameter("WvR", [128, 6, 1536], f16, isOutput=False)
    WPR = nc.declare_dram_parameter("WpR", [128, 12, 768], f16, isOutput=False)
    BPP = nc.declare_dram_parameter("bpp", [128, 6], f32, isOutput=False)
    OUT = nc.declare_dram_parameter("outT", [128, 6, 1024], f32, isOutput=True)
    if dbg:
        DVT = nc.declare_dram_parameter("d_vt", [128, 8, 12, 128], f16, isOutput=True)
        DQH = nc.declare_dram_parameter("d_qh", [128, 1024], f16, isOutput=True)
        DKH = nc.declare_dram_parameter("d_kh", [128, 1024], f16, isOutput=True)
        DE12 = nc.declare_dram_parameter("d_e12", [128, 8, 1024], f16, isOutput=True)
        DZ = nc.declare_dram_parameter("d_z", [128, 12, 1024], f16, isOutput=True)

    # eps correction constant: V = B/128 + KC*A1^2  (mu''^2 folded in)
    KC = (EPS - CONE * CONE * (1.0 - lam) ** 2) * S2 * S2
    lam_is_one = abs(lam - 1.0) < 1e-12

    with tile.TileContext(nc) as tc:
        with tc.tile_pool(name="persist", bufs=1) as persist:
            zbuf = persist.tile([128, 12, 1024], f16)  # LN output, [d, h, n]
            ones = persist.tile([128, 128], f16)
            bpp_sb = persist.tile([128, 6], f32)
            # stt scalars must be [128,1] APs (walrus rejects immediates)
            cst = persist.tile([128, 4], f32)
            mu_c = persist.tile([128, 1], f32)
            wp_all = persist.tile([128, 12, 768], f16)
            with tc.tile_pool(name="longA", bufs=1) as longA:
                xTk = [
                    longA.tile([128, 1024], f16, name=f"xT{k}") for k in range(6)
                ]
                vt = longA.tile([128, 8, 12, 128], f16)

                from contextlib import ExitStack as _ES
                _pools = _ES()
                wqkp = _pools.enter_context(tc.tile_pool(name="wqk", bufs=3))
                qkp = _pools.enter_context(tc.tile_pool(name="qk", bufs=3))
                # one 2-bank ring shared by the qk projection accumulators
                # AND the per-strip stat colsums (their lifetimes interleave)
                aux = _pools.enter_context(
                    tc.tile_pool(name="aux", bufs=2, space="PSUM")
                )

                def emit_qk(h, first=False):
                    """DMA w_q/w_k for head h and project q^T/k^T."""
                    wqh = wqkp.tile([128, 6, 128], f16, tag="wq",
                                    name=f"wqh{h}")
                    wkh = wqkp.tile([128, 6, 128], f16, tag="wk",
                                    name=f"wkh{h}")
                    nc.sync.dma_start(out=wqh[:], in_=WQR[h])
                    nc.sync.dma_start(out=wkh[:], in_=WKR[h])
                    if first:
                        # first head: issue the big input DMAs behind the
                        # head-0 weights so the first matmul starts early
                        for k in range(6):
                            nc.sync.dma_start(out=xTk[k][:], in_=XT[:, k])
                    qh = qkp.tile([128, 1024], f16, tag="q", name=f"qh{h}")
                    kh = qkp.tile([128, 1024], f16, tag="k", name=f"kh{h}")
                    for wt, dst in ((wqh, qh), (wkh, kh)):
                        ps0 = aux.tile([128, 512], f32, tag="aux", name="ps0")
                        ps1 = aux.tile([128, 512], f32, tag="aux", name="ps1")
                        for k in range(6):
                            nc.tensor.matmul(
                                ps0[:], wt[:, k, :], xTk[k][:, 0:512],
                                start=(k == 0), stop=(k == 5),
                            )
                            nc.tensor.matmul(
                                ps1[:], wt[:, k, :], xTk[k][:, 512:1024],
                                start=(k == 0), stop=(k == 5),
                            )
                        nc.vector.tensor_copy(dst[:, 0:512], ps0[:])
                        nc.vector.tensor_copy(dst[:, 512:1024], ps1[:])
                    return qh, kh

                # ---- Phase 1: v~ = x @ Wv_centered + 1/128 into vt ----
                with (
                    tc.tile_pool(name="wv", bufs=1) as wvp,
                    tc.tile_pool(name="vps", bufs=3, space="PSUM") as vps,
                ):
                    next_qk = emit_qk(0, first=True)
                    wvk = [
                        wvp.tile([128, 1536], f16, name=f"wv{k}")
                        for k in range(6)
                    ]
                    for k in range(6):
                        nc.sync.dma_start(out=wvk[k][:], in_=WVR[:, k])
                    nc.sync.dma_start(out=bpp_sb[:], in_=BPP[:])
                    nc.sync.dma_start(out=wp_all[:], in_=WPR[:])
                    nc.vector.memset(ones[:], 1.0)
                    nc.vector.memset(cst[:, 0:1], float(lam * S2))
                    nc.vector.memset(cst[:, 1:2], float(KC))
                    nc.vector.memset(cst[:, 2:3], float(S2))
                    nc.vector.memset(cst[:, 3:4], float(1.0 / 128))
                    nc.vector.memset(mu_c[:], float(CONE * (1.0 - lam) * S2))
                    # cr-major so heads 0-3 (cr=0) finish first
                    for cr in range(3):
                        for t in range(8):
                            ps = vps.tile([128, 512], f32, tag="v",
                                          name=f"vps{cr}_{t}")
                            for k in range(6):
                                nc.tensor.matmul(
                                    ps[:],
                                    xTk[k][:, t * 128: (t + 1) * 128],
                                    wvk[k][:, cr * 512: (cr + 1) * 512],
                                    start=(k == 0),
                                    stop=(k == 5),
                                )
                            nc.scalar.activation(
                                vt[:, t, 4 * cr: 4 * cr + 4, 0:128],
                                ps[:].rearrange("p (h c) -> p h c", c=128),
                                AF.Copy,
                                bias=CONE,
                            )

                # ---- Phase 2: attention per head ----
                with (
                    tc.tile_pool(name="estrip", bufs=3) as ep,
                    tc.tile_pool(name="fin", bufs=2) as fin,
                    tc.tile_pool(name="spool", bufs=2, space="PSUM") as spool,
                    tc.tile_pool(name="avps", bufs=2, space="PSUM") as avps,
                ):

                    def do_scores(h, r, qh, kh):
                        e12 = ep.tile([128, 8, 1024], f16, tag="e")
                        nsl = slice(r * 512, (r + 1) * 512)
                        for m in range(8):
                            msl = slice(m * 128, (m + 1) * 128)
                            # two K=64 matmuls on disjoint PE row groups run
                            # concurrently; they must hit different PSUM banks
                            sp = spool.tile([128, 2, 512], f32, tag="s")
                            nc.tensor.matmul(
                                sp[:, 0, :], kh[0:64, msl], qh[0:64, nsl],
                                start=True, stop=True,
                            )
                            nc.tensor.matmul(
                                sp[:, 1, :], kh[64:128, msl], qh[64:128, nsl],
                                start=True, stop=True,
                            )
                            nc.scalar.activation(
                                e12[:, m, :].rearrange("p (a b) -> p a b", a=2),
                                sp[:],
                                AF.Exp,
                                scale=SCALE,
                            )
                        return e12

                    def do_av(h, r, e12):
                        """AV with v stationary + colsums + per-strip stats
                        through V (variance); returns tiles for the deferred
                        per-head R/apply."""
                        nsl = slice(r * 512, (r + 1) * 512)
                        usb = []
                        for c2 in range(2):
                            ups = avps.tile([128, 512], f32, tag="u")
                            csl = slice(c2 * 512, (c2 + 1) * 512)
                            for m in range(8):
                                nc.tensor.matmul(
                                    ups[:],
                                    vt[:, m, h, :],
                                    e12[:, m, csl],
                                    start=(m == 0),
                                    stop=(m == 7),
                                    skip_group_check=True,
                                )
                            u = fin.tile([128, 512], f16, tag=f"u{c2}")
                            nc.vector.tensor_copy(u[:], ups[:])
                            usb.append(u)
                        a1 = aux.tile([128, 512], f32, tag="aux", name="a1")
                        a2 = aux.tile([128, 512], f32, tag="aux", name="a2")
                        nc.tensor.matmul(a1[:], ones[:], usb[0][:],
                                         start=True, stop=True)
                        nc.tensor.matmul(a2[:], ones[:], usb[1][:],
                                         start=True, stop=True)
                        r2 = fin.tile([128, 512], f32, tag="r2")
                        nc.vector.reciprocal(r2[:], a2[:])
                        # A1 -> SBUF: a DVE op may read PSUM through at most
                        # one port, so K*A1^2 must square the SBUF copy
                        a1s = fin.tile([128, 512], f32, tag="a1s")
                        nc.vector.tensor_copy(a1s[:], a1[:])
                        rho = fin.tile([128, 512], f16, tag="rho")
                        nc.vector.scalar_tensor_tensor(
                            rho[:], a1[:], cst[:, 0:1], r2[:],
                            op0=OP.mult, op1=OP.mult,
                        )
                        tk = fin.tile([128, 512], f32, tag="tk")
                        nc.vector.scalar_tensor_tensor(
                            tk[:], a1s[:], cst[:, 1:2], a1s[:],
                            op0=OP.mult, op1=OP.mult,
                        )
                        tmp = fin.tile([128, 512], f16, tag="tmp")
                        nc.vector.tensor_tensor(
                            tmp[:], rho[:], usb[1][:], op=OP.mult
                        )
                        zst = zbuf[:, h, nsl]
                        nc.vector.scalar_tensor_tensor(
                            zst, usb[0][:], cst[:, 2:3], tmp[:],
                            op0=OP.mult, op1=OP.subtract,
                        )
                        sq = fin.tile([128, 512], f16, tag="sq")
                        nc.vector.tensor_tensor(sq[:], zst, zst, op=OP.mult)
                        return sq, tk, a1s

                    def do_tail(h, pend):
                        """B-colsums + rsqrt + apply for head h (deferred)."""
                        vv = fin.tile([128, 2, 512], f32, tag="vv")
                        for r, (sq, tk, a1s) in enumerate(pend):
                            b = aux.tile([128, 512], f32, tag="aux", name="b")
                            nc.tensor.matmul(b[:], ones[:], sq[:],
                                             start=True, stop=True)
                            nc.vector.scalar_tensor_tensor(
                                vv[:, r, :], b[:], cst[:, 3:4], tk[:],
                                op0=OP.mult, op1=OP.add,
                            )
                        lnv = fin.tile([128, 2, 512], f32, tag="lnv")
                        nc.scalar.activation(lnv[:], vv[:], AF.Ln)
                        rr = fin.tile([128, 2, 512], f16, tag="rr")
                        nc.scalar.activation(rr[:], lnv[:], AF.Exp, scale=-0.5)
                        for r, (sq, tk, a1s) in enumerate(pend):
                            nsl = slice(r * 512, (r + 1) * 512)
                            zst = zbuf[:, h, nsl]
                            nc.vector.tensor_tensor(
                                zst, zst, rr[:, r, :], op=OP.mult
                            )
                            if not lam_is_one:
                                # subtract mu''*R = c*(1-lam)*2^-s*A1*R
                                mr = fin.tile([128, 512], f16, tag="mr")
                                nc.vector.tensor_scalar(
                                    mr[:], a1s[:], mu_c[:, 0:1], None,
                                    op0=OP.mult,
                                )
                                nc.vector.tensor_tensor(
                                    mr[:], mr[:], rr[:, r, :], op=OP.mult
                                )
                                nc.vector.tensor_tensor(
                                    zst, zst, mr[:], op=OP.subtract
                                )

                    pend = None  # (h, [(sq, tk, a1s) per strip])
                    for h in range(12):
                        qh, kh = next_qk
                        e0 = do_scores(h, 0, qh, kh)
                        if pend is not None:
                            do_tail(*pend)
                        e1 = do_scores(h, 1, qh, kh)
                        if h + 1 < 12:
                            next_qk = emit_qk(h + 1)
                        p0 = do_av(h, 0, e0)
                        p1 = do_av(h, 1, e1)
                        pend = (h, [p0, p1])
                        if dbg and h == 0:
                            nc.sync.dma_start(out=DQH[:], in_=qh[:])
                            nc.sync.dma_start(out=DKH[:], in_=kh[:])
                            nc.sync.dma_start(out=DE12[:], in_=e0[:])
                    do_tail(*pend)
                    if dbg:
                        nc.sync.dma_start(out=DVT[:], in_=vt[:])

                _pools.close()

            # longA (xT, vt) released here.
            if dbg:
                nc.sync.dma_start(out=DZ[:], in_=zbuf[:])
            # ---- Phase 3: final projection ----
            with tc.tile_pool(name="tail", bufs=1) as tailp:
                fout = tailp.tile([128, 6, 1024], f32)
                with tc.tile_pool(name="fps", bufs=1, space="PSUM") as fps:
                    # mc-groups of 3 so each group's 3x2 accumulators fit in
                    # 6 PSUM banks
                    for g in range(2):
                        fs = {}
                        for mc in range(3 * g, 3 * g + 3):
                            for nr2 in range(2):
                                fs[(mc, nr2)] = fps.tile(
                                    [128, 512], f32, tag=f"f{mc % 3}_{nr2}",
                                    name=f"fpsum{mc}_{nr2}",
                                )
                        for k in range(12):
                            for mc in range(3 * g, 3 * g + 3):
                                for nr2 in range(2):
                                    nc.tensor.matmul(
                                        fs[(mc, nr2)][:],
                                        wp_all[:, k, mc * 128: (mc + 1) * 128],
                                        zbuf[:, k, nr2 * 512: (nr2 + 1) * 512],
                                        start=(k == 0),
                                        stop=(k == 11),
                                    )
                        for mc in range(3 * g, 3 * g + 3):
                            for nr2 in range(2):
                                nsl2 = slice(nr2 * 512, (nr2 + 1) * 512)
                                nc.vector.tensor_scalar(
                                    fout[:, mc, nsl2],
                                    fs[(mc, nr2)][:],
                                    bpp_sb[:, mc: mc + 1],
                                    None,
                                    op0=OP.add,
                                )
                                nc.sync.dma_start(
                                    out=OUT[:, mc, nsl2],
                                    in_=fout[:, mc, nsl2],
                                )

    nc.compile()
    return nc


def _host_prep(x, Wq, Wk, Wv, gamma, beta, Wp, bp):
    x = np.ascontiguousarray(np.asarray(x, np.float32))
    Wq = np.asarray(Wq, np.float32)
    Wk = np.asarray(Wk, np.float32)
    Wv = np.asarray(Wv, np.float32)
    Wp = np.asarray(Wp, np.float32)
    bp = np.asarray(bp, np.float32)
    gamma = np.asarray(gamma, np.float32)
    beta = np.asarray(beta, np.float32)

    # xT per batch: [128, 6, 1024] with [p, k, n] = x[b, n, k*128+p]
    xTr = np.ascontiguousarray(
        x.transpose(0, 2, 1).reshape(B, 6, 128, N).transpose(0, 2, 1, 3)
    ).astype(np.float16)

    # W[qk]R: [12, 128, 6, 128] with [h, p, k, c] = W[k*128+p, h*128+c]
    def wqk_r(W):
        return np.ascontiguousarray(
            W.reshape(6, 128, 12, 128).transpose(2, 1, 0, 3)
        )

    WqR = wqk_r(Wq).astype(np.float16)
    WkR = wqk_r(Wk).astype(np.float16)
    # center Wv columns within each head block so each value row of v~
    # sums to 128*CONE = 1 after the on-device +CONE offset
    Wv_h = Wv.reshape(C, H, TD)
    Wv_c = (Wv_h - Wv_h.mean(axis=2, keepdims=True)).reshape(C, 2 * C)
    # WvR: [128, 6, 1536] with [p, k, c] = Wv_c[k*128+p, c]
    WvR = np.ascontiguousarray(
        Wv_c.reshape(6, 128, 2 * C).transpose(1, 0, 2)
    ).astype(np.float16)
    # Fold gamma and the (1 - lambda_init) scale into Wp; beta into the bias.
    gfull = np.tile(gamma, H)  # [1536]
    Wpg = Wp * (OUT_SCALE * gfull)[:, None]
    bpp = bp + OUT_SCALE * (np.tile(beta, H) @ Wp)
    WpR = np.ascontiguousarray(
        Wpg.reshape(12, 128, C).transpose(1, 0, 2)
    ).astype(np.float16)
    bppR = np.ascontiguousarray(bpp.reshape(6, 128).T)  # [128, 6]
    return xTr, WqR, WkR, WvR, WpR, bppR


def kernel(x, Wq, Wk, Wv, lam, gamma, beta, Wp, bp):
    global LAST_EXEC_NS
    import os

    from concourse.bass_utils import run_bass_kernel_spmd

    lam_f = float(np.asarray(lam))
    xTr, WqR, WkR, WvR, WpR, bppR = _host_prep(
        x, Wq, Wk, Wv, gamma, beta, Wp, bp
    )

    dbg = bool(os.environ.get("BASS_KERNEL_DBG"))
    key = (lam_f, dbg)
    if key not in _BUILD_CACHE:
        _BUILD_CACHE[key] = _build(lam_f, dbg=dbg)
    nc = _BUILD_CACHE[key]

    in_maps = [
        {
            "xT": xTr[b],
            "WqR": WqR,
            "WkR": WkR,
            "WvR": WvR,
            "WpR": WpR,
            "bpp": bppR,
        }
        for b in range(B)
    ]

    trace = bool(os.environ.get("BASS_KERNEL_TRACE"))
    if trace:
        from concourse import bass_utils as _bu

        _bu.upload_artifacts = lambda tmpdir: "local://" + tmpdir
    res = run_bass_kernel_spmd(
        nc, in_maps, list(range(B)), trace=trace,
        **({"trace_cores": list(range(B))} if trace else {}),
    )
    LAST_EXEC_NS = res.exec_time_ns
    if dbg:
        kernel.dbg_results = res.results

    out = np.empty((B, N, C), np.float32)
    for b in range(B):
        outT = res.results[b]["outT"]  # [128, 6, 1024]
        out[b] = outT.transpose(2, 1, 0).reshape(N, C)
    return out
